# revision 26
# baseline (speedup 1.0000x reference)
"""Trainium2 Bass kernel for a CAM (channel-attention) module.

Computes, per batch b:
    E = X @ X^T                      (C x C channel energy, X = x[b] in R^{C x L})
    A = softmax(rowmax(E) - E)       (== softmax(-E) row-wise, stabilized)
    y[b] = gamma * (A @ X) + x[b]

Shapes: x [32, 512, 4096] f32, gamma [1] f32.  Data-parallel over batch:
8 NeuronCores x 4 batches each.  No cross-core communication.

Device-side algorithm per batch (all matmuls on the PE systolic array):
  - mm1: E chunks [128c, 512d] in fp8-e4m3 DoubleRow (2 contraction rows
    per PE cell -> 2x column rate) from a host-quantized transposed copy
    xt8 [L, C].  Upper-triangle block-columns only; the lower blocks are
    PE-transposed from earlier chunks (E is symmetric; block copies are
    staged to SBUF by DVE).
  - softmax: row-min of E (DVE, from PSUM), ScalarE Exp(-E + min) -> P
    bf16 with row-sum s via accum_out.  r = 1/s and diag(s) on DVE.
  - PT: PE transposes of P; the PSUM->SBUF copies run on ScalarE as
    activation-Copy with per-partition scale = gamma, casting to fp8.
    So mm2 computes U = gamma * P @ X directly.
  - mm2: DoubleRow fp8 over the C=512 contraction (2 steps) against a
    host-quantized xb8 [C, L].  For 3 of every 8 output tiles an extra
    bf16 matmul accumulates diag(s) @ X into the same PSUM tile
    (-> gamma*U + s*x) and ScalarE finishes with Copy(scale=1/s); the
    other 5 are finished on DVE as scalar_tensor_tensor (1/s * U + x)
    against a bf16 xb16.  This splits the PSUM drain across both
    engines.  y is written bf16 and upcast on the host.

Software pipelining: mm2 of batch b-1 is emitted interleaved between the
mm1 chunks of batch b, so the PE fills epilogue-drain stalls with mm1
work; the pipeline tail borrows the idle e-PSUM banks for deeper
buffering.  All dram inputs are host-swizzled partition-major so every
load is one fully-contiguous DMA.
"""

import numpy as np
import ml_dtypes

B, C, L = 32, 512, 4096
N_CORES = 8
BPC = B // N_CORES  # batches per core

FP8_NP = ml_dtypes.float8_e4m3  # TRN float8e4 (bias 7, max 240)

_CACHE: dict = {}


def build_nc(bpc: int = BPC, repeat: int = 1, hw_loop: int = 0, fold_jj: int = 3):
    from contextlib import ExitStack

    import concourse.bass as bass  # noqa: F401  (registers engines)
    import concourse.tile as tile
    from concourse import bacc, masks, mybir

    f32 = mybir.dt.float32
    bf16 = mybir.dt.bfloat16
    fp8 = mybir.dt.float8e4
    AX = mybir.AxisListType
    OP = mybir.AluOpType
    ACT = mybir.ActivationFunctionType
    DR = mybir.MatmulPerfMode.DoubleRow

    NCC = C // 128  # 4 c-chunks (partition blocks of C)
    NLT = L // 128  # 32 l-tiles
    NDR = NLT // 2  # 16 DoubleRow contraction steps for mm1
    NJ = L // 512  # 8 mm2 output column chunks per c-chunk

    nc = bacc.Bacc("TRN2", target_bir_lowering=False, debug=False, num_devices=N_CORES)
    # host pre-swizzled, partition-major layouts -> fully contiguous DMA loads
    xt8d = nc.dram_tensor("xt8", [bpc, 128, NLT, C], fp8, kind="ExternalInput")
    xb8d = nc.dram_tensor("xb8", [bpc, 128, 2, 2, L], fp8, kind="ExternalInput")
    xb16d = nc.dram_tensor("xb16", [bpc, 128, NCC, L], bf16, kind="ExternalInput")
    gd = nc.dram_tensor("gamma", [1, 1], f32, kind="ExternalInput")
    yd = nc.dram_tensor("y", [bpc, C, L], bf16, kind="ExternalOutput")

    with tile.TileContext(nc) as tc, ExitStack() as ctx:
        const = ctx.enter_context(tc.tile_pool(name="const", bufs=1))
        xt_pool = ctx.enter_context(tc.tile_pool(name="xt", bufs=2))
        xb8_pool = ctx.enter_context(tc.tile_pool(name="xb8", bufs=2))
        xb16_pool = ctx.enter_context(tc.tile_pool(name="xb16", bufs=2))
        prow_pool = ctx.enter_context(tc.tile_pool(name="prow", bufs=5))
        pt_pool = ctx.enter_context(tc.tile_pool(name="pt", bufs=4))
        eblk_pool = ctx.enter_context(tc.tile_pool(name="eblk", bufs=6))
        out_pool = ctx.enter_context(tc.tile_pool(name="out", bufs=5))
        st_pool = ctx.enter_context(tc.tile_pool(name="stats", bufs=16))
        stg_pool = ctx.enter_context(tc.tile_pool(name="stg", bufs=6))
        e_psum = ctx.enter_context(tc.tile_pool(name="e_ps", bufs=2, space="PSUM"))
        t_psum = ctx.enter_context(tc.tile_pool(name="t_ps", bufs=2, space="PSUM"))
        u_psum = ctx.enter_context(tc.tile_pool(name="u_ps", bufs=4, space="PSUM"))

        pad = stg_pool.tile([128, 512], bf16, name="pad", tag="pad")  # noqa: F841
        identity = const.tile([128, 128], bf16)
        masks.make_identity(nc, identity[:])
        identity_f = const.tile([128, 128], f32)
        masks.make_identity(nc, identity_f[:])
        g_sb = const.tile([1, 1], f32)
        nc.sync.dma_start(g_sb[:], gd.ap())
        gamma_bc = const.tile([128, 1], f32)
        nc.gpsimd.partition_broadcast(gamma_bc[:], g_sb[:])

        # fold_pat[jj]: True -> diag(s)-fold + ScalarE Copy(1/s); False ->
        # DVE STT (1/s * U + x).  S,V alternating then V-tail balances the
        # two engines' total epilogue time.
        fold_pat = [False] * NJ
        for q in range(fold_jj):
            fold_pat[2 * q] = True

        def emit_mm2_quarter(prev, m, tail=False):
            """One m-chunk of mm2 (+epilogue +store) for a previous batch."""
            b_p, pt_p, diag_p, r_p, xb8_p, xb16_p, o_p = prev
            o_t = o_p[m]
            for g in range(NJ // 2):
                # in the pipeline tail (no mm1 to interleave) borrow the idle
                # e_psum banks for deeper PSUM double-buffering
                if tail and g % 2 == 1:
                    us = [
                        e_psum.tile([128, 512], f32, name="e_t", tag="e_t")
                        for _ in range(2)
                    ]
                else:
                    us = [
                        u_psum.tile([128, 512], f32, name="u_t", tag="u_t")
                        for _ in range(2)
                    ]
                for ip in range(2):
                    for q in range(2):
                        jj = g * 2 + q
                        nc.tensor.matmul(
                            us[q][:],
                            lhsT=pt_p[ip][:, :, m * 128 : (m + 1) * 128],
                            rhs=xb8_p[:, ip, :, jj * 512 : (jj + 1) * 512],
                            perf_mode=DR,
                            start=(ip == 0),
                            stop=(ip == 1 and not fold_pat[jj]),
                        )
                for q in range(2):
                    jj = g * 2 + q
                    sl_x = slice(jj * 512, (jj + 1) * 512)
                    sl_o = sl_x
                    if fold_pat[jj]:
                        nc.tensor.matmul(
                            us[q][:],
                            lhsT=diag_p[m][:],
                            rhs=xb16_p[:, m, sl_x],
                            start=False,
                            stop=True,
                        )
                        nc.scalar.activation(
                            o_t[:, sl_o], us[q][:], ACT.Copy, scale=r_p[m][:]
                        )
                    else:
                        nc.vector.scalar_tensor_tensor(
                            o_t[:, sl_o],
                            us[q][:],
                            r_p[m][:],
                            xb16_p[:, m, sl_x],
                            op0=OP.mult,
                            op1=OP.add,
                        )
                if g == NJ // 4 - 1:
                    nc.gpsimd.dma_start(
                        yd.ap()[b_p, m * 128 : (m + 1) * 128, : L // 2],
                        o_t[:, : L // 2],
                    )
            nc.gpsimd.dma_start(
                yd.ap()[b_p, m * 128 : (m + 1) * 128, L // 2 :], o_t[:, L // 2 :]
            )

        loop_cm = tc.For_i(0, hw_loop, 1) if hw_loop else None
        if loop_cm is not None:
            ctx.enter_context(loop_cm)
        prev = None
        for b_rep in range(bpc * repeat):
            b = b_rep % bpc
            # --- loads (contiguous: dram layouts are partition-major) ---
            xt_t = xt_pool.tile([128, NLT, C], fp8, name="xt_t", tag="xt_t")
            nc.sync.dma_start(xt_t[:], xt8d.ap()[b])
            xb8_t = xb8_pool.tile([128, 2, 2, L], fp8, name="xb8_t", tag="xb8_t")
            nc.sync.dma_start(xb8_t[:], xb8d.ap()[b])
            xb16_t = xb16_pool.tile([128, NCC, L], bf16, name="xb16_t", tag="xb16_t")
            nc.sync.dma_start(xb16_t[:], xb16d.ap()[b])

            # --- mm1 (upper-triangle block-columns only; E is symmetric) ---
            psc_sb = []
            r_ts = []
            diag_ts = []
            eblk_sb = {}  # (dc, m) -> SBUF copy of E[dc][:, m-block]
            for m in range(NCC):
                e_t = e_psum.tile([128, C], f32, name="e_t", tag="e_t")
                mm0 = None
                for i in range(NDR):
                    mm = nc.tensor.matmul(
                        e_t[:, m * 128 :],
                        lhsT=xt_t[:, 2 * i : 2 * i + 2, m * 128 : (m + 1) * 128],
                        rhs=xt_t[:, 2 * i : 2 * i + 2, m * 128 :],
                        perf_mode=DR,
                        start=(i == 0),
                        stop=(i == NDR - 1),
                    )
                    if i == 0:
                        mm0 = mm
                # fill columns [0:m*128] by transposing earlier chunks' blocks.
                # start=False so the accumulation group's has_written clear is
                # not re-triggered; the explicit dep keeps each transpose after
                # that group's first matmul.
                for dc in range(m):
                    tr = nc.tensor.matmul(
                        e_t[:, dc * 128 : (dc + 1) * 128],
                        lhsT=eblk_sb.pop((dc, m))[:],
                        rhs=identity_f[:],
                        is_transpose=True,
                        start=False,
                        stop=True,
                        skip_group_check=True,
                    )
                    tile.add_dep_helper(
                        tr.ins, mm0.ins, reason="transpose after bank clear"
                    )
                # stage upper blocks needed by later chunks before e_t is freed
                for mc in range(m + 1, NCC):
                    blk = eblk_pool.tile([128, 128], f32, name="eblk", tag="eblk")
                    nc.vector.tensor_copy(blk[:], e_t[:, mc * 128 : (mc + 1) * 128])
                    eblk_sb[(m, mc)] = blk
                m_t = st_pool.tile([128, 1], f32)
                nc.vector.tensor_reduce(m_t[:], e_t[:], axis=AX.X, op=OP.min)
                p_t = prow_pool.tile([128, C], bf16)
                s_t = st_pool.tile([128, 1], f32)
                nc.scalar.activation(
                    p_t[:], e_t[:], ACT.Exp, bias=m_t[:], scale=-1.0, accum_out=s_t[:]
                )
                r_t = st_pool.tile([128, 1], f32, name="r_t", tag="r_t", bufs=8)
                nc.vector.reciprocal(r_t[:], s_t[:])
                d_t = st_pool.tile([128, 128], bf16, name="d_t", tag="d_t", bufs=8)
                nc.vector.tensor_scalar_mul(d_t[:], identity[:], s_t[:])
                r_ts.append(r_t)
                diag_ts.append(d_t)
                psc_sb.append(p_t)
                # interleave one mm2 m-chunk of the previous batch between
                # mm1 chunks so the PE fills epilogue-drain gaps
                if prev is not None:
                    emit_mm2_quarter(prev, m)

            # --- transpose P -> PT tiles [128 d, 2, C], scaled by gamma, fp8 ---
            pt_sb = [
                pt_pool.tile([128, 2, C], fp8, name="pt_sb", tag="pt_sb")
                for _ in range(NCC // 2)
            ]
            for m in range(NCC):
                for i in range(NCC):
                    tp = t_psum.tile([128, 128], bf16)
                    nc.tensor.transpose(
                        tp[:], psc_sb[m][:, i * 128 : (i + 1) * 128], identity[:]
                    )
                    dst = pt_sb[i // 2][:, i % 2, m * 128 : (m + 1) * 128]
                    nc.scalar.activation(dst, tp[:], ACT.Copy, scale=gamma_bc[:])

            o_sb = [
                out_pool.tile([128, L], bf16, name="o_t", tag="o_t")
                for _ in range(NCC)
            ]
            prev = (b, pt_sb, diag_ts, r_ts, xb8_t, xb16_t, o_sb)

        # pipeline tail: mm2 of the last batch
        for m in range(NCC):
            emit_mm2_quarter(prev, m, tail=True)

    nc.compile()
    return nc


def _get_nc():
    if "nc" not in _CACHE:
        _CACHE["nc"] = build_nc(BPC)
    return _CACHE["nc"]


def _prep_inputs(x: np.ndarray, gamma: np.ndarray):
    x = np.ascontiguousarray(np.asarray(x, dtype=np.float32))
    gamma = np.asarray(gamma, dtype=np.float32).reshape(1, 1)
    # partition-major swizzles so every device DMA load is contiguous:
    # xb16 [b, p, m, l] with c = m*128 + p
    xb16 = np.ascontiguousarray(
        x.reshape(B, 4, 128, L).transpose(0, 2, 1, 3)
    ).astype(ml_dtypes.bfloat16)
    # xb8 [b, p, i, j, l] with c = i*256 + j*128 + p  (DoubleRow k-pairing)
    xb8 = np.ascontiguousarray(
        x.reshape(B, 2, 2, 128, L).transpose(0, 3, 1, 2, 4)
    ).astype(FP8_NP)
    # xt8 [b, p, n, c] with l = n*128 + p
    xt8 = np.ascontiguousarray(
        x.transpose(0, 2, 1).reshape(B, L // 128, 128, C).transpose(0, 2, 1, 3)
    ).astype(FP8_NP)
    in_maps = []
    for c in range(N_CORES):
        sl = slice(c * BPC, (c + 1) * BPC)
        in_maps.append(
            {
                "xt8": np.ascontiguousarray(xt8[sl]),
                "xb8": np.ascontiguousarray(xb8[sl]),
                "xb16": np.ascontiguousarray(xb16[sl]),
                "gamma": gamma,
            }
        )
    return in_maps


def kernel(x: np.ndarray, gamma: np.ndarray) -> np.ndarray:
    from concourse.bass_utils import run_bass_kernel_spmd

    nc = _get_nc()
    in_maps = _prep_inputs(x, gamma)
    res = run_bass_kernel_spmd(nc, in_maps, core_ids=list(range(N_CORES)))
    y = np.concatenate([res.results[c]["y"] for c in range(N_CORES)], axis=0)
    return y.astype(np.float32)


def _make_exec_jit(nc, in_specs_names, out_shape, out_dtype=None):
    """One-bass_exec jit over 8 cores, mirroring run_bass_via_pjrt."""
    import jax
    from jax.sharding import Mesh, PartitionSpec
    from jax.experimental.shard_map import shard_map
    from concourse.bass2jax import (
        _bass_exec_p,
        install_neuronx_cc_hook,
        partition_id_tensor,
    )

    install_neuronx_cc_hook()
    out_aval = jax.core.ShapedArray(out_shape, out_dtype or np.float32)
    out_name = in_specs_names[-1]

    def body(*args):
        outs = _bass_exec_p.bind(
            *args,
            partition_id_tensor(),
            out_avals=(out_aval,),
            in_names=tuple(in_specs_names) + ("partition_id",),
            out_names=(out_name,),
            lowering_input_output_aliases=(),
            sim_require_finite=True,
            sim_require_nnan=True,
            nc=nc,
        )
        return outs[0]

    mesh = Mesh(np.asarray(jax.devices()[:N_CORES]), ("core",))
    spec = PartitionSpec("core")
    jitted = jax.jit(
        shard_map(
            body,
            mesh=mesh,
            in_specs=(spec,) * len(in_specs_names),
            out_specs=spec,
            check_rep=False,
        ),
        keep_unused=True,
    )
    sharding = jax.sharding.NamedSharding(mesh, spec)
    return jitted, sharding


def _build_tiny_nc():
    """Minimal kernel with the same call structure, for dispatch-floor calibration."""
    import concourse.tile as tile
    from concourse import bacc, mybir

    f32 = mybir.dt.float32
    nc = bacc.Bacc("TRN2", target_bir_lowering=False, debug=False, num_devices=N_CORES)
    ad = nc.dram_tensor("a", [128, 128], f32, kind="ExternalInput")
    bd = nc.dram_tensor("bout", [128, 128], f32, kind="ExternalOutput")
    with tile.TileContext(nc) as tc:
        with tc.tile_pool(name="p", bufs=1) as pool:
            t = pool.tile([128, 128], f32)
            nc.sync.dma_start(t[:], ad.ap())
            nc.sync.dma_start(bd.ap(), t[:])
    nc.compile()
    return nc


def measure_hw_time(x: np.ndarray, gamma: np.ndarray, calls: int = 30, reps: int = 5):
    """Estimate per-NEFF device time: loop a cached jit on device-resident
    inputs, subtract the dispatch floor measured with a near-empty kernel.

    Returns (exec_ns_estimate, per_call_big_ns, per_call_tiny_ns)."""
    import time

    import jax

    nc = _get_nc()
    in_maps = _prep_inputs(x, gamma)

    names = ["xt8", "xb8", "xb16", "gamma", "y"]
    jit_big, sh = _make_exec_jit(nc, names, (BPC, C, L), ml_dtypes.bfloat16)
    args = [
        np.concatenate([m[k] for m in in_maps], axis=0) for k in names[:-1]
    ] + [np.zeros((B, C, L), ml_dtypes.bfloat16)]
    big_args = [jax.device_put(a, sh) for a in args]

    tiny = _CACHE.get("tiny_nc")
    if tiny is None:
        tiny = _CACHE["tiny_nc"] = _build_tiny_nc()
    jit_tiny, sh2 = _make_exec_jit(tiny, ["a", "bout"], (128, 128))
    a_g = np.zeros((N_CORES * 128, 128), np.float32)
    tiny_args = [jax.device_put(a, sh2) for a in (a_g, np.zeros_like(a_g))]

    jax.block_until_ready(jit_big(*big_args))
    jax.block_until_ready(jit_tiny(*tiny_args))

    def per_call(f, args):
        best = np.inf
        for _ in range(reps):
            t0 = time.perf_counter()
            for _ in range(calls):
                out = f(*args)
            jax.block_until_ready(out)
            best = min(best, (time.perf_counter() - t0) / calls)
        return best * 1e9

    t_tiny = per_call(jit_tiny, tiny_args)
    t_big = per_call(jit_big, big_args)
    return t_big - t_tiny, t_big, t_tiny


if __name__ == "__main__":
    rng = np.random.default_rng(0)
    x = rng.standard_normal((B, C, L), dtype=np.float32)
    gamma = np.zeros((1,), np.float32)
    y = kernel(x, gamma)
    err = np.abs(y - x).max() / np.abs(x).max()
    print("gamma=0 rel err (bf16 roundtrip expected):", err)
    ns, t1, t0 = measure_hw_time(x, gamma)
    print(f"HW exec time: {ns:.0f} ns  (single-call wall {t1:.0f} ns)")


# revision 28
# speedup vs baseline: 1.0307x; 1.0307x over previous
"""Trainium2 Bass kernel for a CAM (channel-attention) module.

Computes, per batch b:
    E = X @ X^T                      (C x C channel energy, X = x[b] in R^{C x L})
    A = softmax(rowmax(E) - E)       (== softmax(-E) row-wise, stabilized)
    y[b] = gamma * (A @ X) + x[b]

Shapes: x [32, 512, 4096] f32, gamma [1] f32.  Data-parallel over batch:
8 NeuronCores x 4 batches each.  No cross-core communication.

Device-side algorithm per batch (all matmuls on the PE systolic array):
  - mm1: E chunks [128c, 512d] in fp8-e4m3 DoubleRow (2 contraction rows
    per PE cell -> 2x column rate) from a host-quantized transposed copy
    xt8 [L, C].  Upper-triangle block-columns only; the lower blocks are
    PE-transposed from earlier chunks (E is symmetric; block copies are
    staged to SBUF by DVE).
  - softmax: row-min of E (DVE, from PSUM), ScalarE Exp(-E + min) -> P
    bf16 with row-sum s via accum_out.  r = 1/s and diag(s) on DVE.
  - PT: PE transposes of P; the PSUM->SBUF copies run on ScalarE as
    activation-Copy with per-partition scale = gamma, casting to fp8.
    So mm2 computes U = gamma * P @ X directly.
  - mm2: DoubleRow fp8 over the C=512 contraction (2 steps) against a
    host-quantized xb8 [C, L].  For 3 of every 8 output tiles an extra
    bf16 matmul accumulates diag(s) @ X into the same PSUM tile
    (-> gamma*U + s*x) and ScalarE finishes with Copy(scale=1/s); the
    other 5 are finished on DVE as scalar_tensor_tensor (1/s * U + x)
    against a bf16 xb16.  This splits the PSUM drain across both
    engines.  y is written bf16 and upcast on the host.

Software pipelining: mm2 of batch b-1 is emitted interleaved between the
mm1 chunks of batch b, so the PE fills epilogue-drain stalls with mm1
work; the pipeline tail borrows the idle e-PSUM banks for deeper
buffering.  All dram inputs are host-swizzled partition-major so every
load is one fully-contiguous DMA.
"""

import numpy as np
import ml_dtypes

B, C, L = 32, 512, 4096
N_CORES = 8
BPC = B // N_CORES  # batches per core

FP8_NP = ml_dtypes.float8_e4m3  # TRN float8e4 (bias 7, max 240)

_CACHE: dict = {}


def build_nc(bpc: int = BPC, repeat: int = 1, hw_loop: int = 0, fold_jj: int = 3):
    from contextlib import ExitStack

    import concourse.bass as bass  # noqa: F401  (registers engines)
    import concourse.tile as tile
    from concourse import bacc, masks, mybir

    f32 = mybir.dt.float32
    bf16 = mybir.dt.bfloat16
    fp8 = mybir.dt.float8e4
    AX = mybir.AxisListType
    OP = mybir.AluOpType
    ACT = mybir.ActivationFunctionType
    DR = mybir.MatmulPerfMode.DoubleRow

    NCC = C // 128  # 4 c-chunks (partition blocks of C)
    NLT = L // 128  # 32 l-tiles
    NDR = NLT // 2  # 16 DoubleRow contraction steps for mm1
    NJ = L // 512  # 8 mm2 output column chunks per c-chunk

    nc = bacc.Bacc("TRN2", target_bir_lowering=False, debug=False, num_devices=N_CORES)
    # host pre-swizzled, partition-major layouts -> fully contiguous DMA loads
    xt8d = nc.dram_tensor("xt8", [bpc, 128, NLT, C], fp8, kind="ExternalInput")
    xb8d = nc.dram_tensor("xb8", [bpc, 128, 2, 2, L], fp8, kind="ExternalInput")
    xb16d = nc.dram_tensor("xb16", [bpc, 128, NCC, L], bf16, kind="ExternalInput")
    gd = nc.dram_tensor("gamma", [1, 1], f32, kind="ExternalInput")
    yd = nc.dram_tensor("y", [bpc, C, L], bf16, kind="ExternalOutput")

    with tile.TileContext(nc) as tc, ExitStack() as ctx:
        const = ctx.enter_context(tc.tile_pool(name="const", bufs=1))
        xt_pool = ctx.enter_context(tc.tile_pool(name="xt", bufs=2))
        xb8_pool = ctx.enter_context(tc.tile_pool(name="xb8", bufs=2))
        xb16_pool = ctx.enter_context(tc.tile_pool(name="xb16", bufs=2))
        prow_pool = ctx.enter_context(tc.tile_pool(name="prow", bufs=5))
        pt_pool = ctx.enter_context(tc.tile_pool(name="pt", bufs=4))
        eblk_pool = ctx.enter_context(tc.tile_pool(name="eblk", bufs=6))
        out_pool = ctx.enter_context(tc.tile_pool(name="out", bufs=5))
        st_pool = ctx.enter_context(tc.tile_pool(name="stats", bufs=16))
        stg_pool = ctx.enter_context(tc.tile_pool(name="stg", bufs=6))
        e_psum = ctx.enter_context(tc.tile_pool(name="e_ps", bufs=2, space="PSUM"))
        t_psum = ctx.enter_context(tc.tile_pool(name="t_ps", bufs=2, space="PSUM"))
        u_psum = ctx.enter_context(tc.tile_pool(name="u_ps", bufs=4, space="PSUM"))

        pad = stg_pool.tile([128, 512], bf16, name="pad", tag="pad")  # noqa: F841
        identity = const.tile([128, 128], bf16)
        masks.make_identity(nc, identity[:])
        identity_f = const.tile([128, 128], f32)
        masks.make_identity(nc, identity_f[:])
        g_sb = const.tile([1, 1], f32)
        nc.sync.dma_start(g_sb[:], gd.ap())
        gamma_bc = const.tile([128, 1], f32)
        nc.gpsimd.partition_broadcast(gamma_bc[:], g_sb[:])

        # fold_pat[jj]: True -> diag(s)-fold + ScalarE Copy(1/s); False ->
        # DVE STT (1/s * U + x).  S,V alternating then V-tail balances the
        # two engines' total epilogue time.
        fold_pat = [False] * NJ
        for q in range(fold_jj):
            fold_pat[2 * q] = True

        def emit_mm2_quarter(prev, m, tail=False):
            """One m-chunk of mm2 (+epilogue +store) for a previous batch."""
            b_p, pt_p, diag_p, r_p, xb8_p, xb16_p, o_p = prev
            o_t = o_p[m]
            for g in range(NJ // 2):
                # in the pipeline tail (no mm1 to interleave) borrow the idle
                # e_psum banks for deeper PSUM double-buffering
                if tail and g % 2 == 1:
                    us = [
                        e_psum.tile([128, 512], f32, name="e_t", tag="e_t")
                        for _ in range(2)
                    ]
                else:
                    us = [
                        u_psum.tile([128, 512], f32, name="u_t", tag="u_t")
                        for _ in range(2)
                    ]
                for ip in range(2):
                    for q in range(2):
                        jj = g * 2 + q
                        nc.tensor.matmul(
                            us[q][:],
                            lhsT=pt_p[ip][:, :, m * 128 : (m + 1) * 128],
                            rhs=xb8_p[:, ip, :, jj * 512 : (jj + 1) * 512],
                            perf_mode=DR,
                            start=(ip == 0),
                            stop=(ip == 1 and not fold_pat[jj]),
                        )
                for q in range(2):
                    jj = g * 2 + q
                    sl_x = slice(jj * 512, (jj + 1) * 512)
                    sl_o = sl_x
                    if fold_pat[jj]:
                        nc.tensor.matmul(
                            us[q][:],
                            lhsT=diag_p[m][:],
                            rhs=xb16_p[:, m, sl_x],
                            start=False,
                            stop=True,
                        )
                        nc.scalar.activation(
                            o_t[:, sl_o], us[q][:], ACT.Copy, scale=r_p[m][:]
                        )
                    else:
                        nc.vector.scalar_tensor_tensor(
                            o_t[:, sl_o],
                            us[q][:],
                            r_p[m][:],
                            xb16_p[:, m, sl_x],
                            op0=OP.mult,
                            op1=OP.add,
                        )
                if g == NJ // 4 - 1:
                    nc.gpsimd.dma_start(
                        yd.ap()[b_p, m * 128 : (m + 1) * 128, : L // 2],
                        o_t[:, : L // 2],
                    )
            nc.gpsimd.dma_start(
                yd.ap()[b_p, m * 128 : (m + 1) * 128, L // 2 :], o_t[:, L // 2 :]
            )

        loop_cm = tc.For_i(0, hw_loop, 1) if hw_loop else None
        if loop_cm is not None:
            ctx.enter_context(loop_cm)
        prev = None
        for b_rep in range(bpc * repeat):
            b = b_rep % bpc
            # --- loads (contiguous: dram layouts are partition-major) ---
            xt_t = xt_pool.tile([128, NLT, C], fp8, name="xt_t", tag="xt_t")
            # two-part load: mm1's first DR steps start after the first half
            nc.sync.dma_start(
                xt_t[:, : NLT // 2, :], xt8d.ap()[b, :, : NLT // 2, :]
            )
            nc.sync.dma_start(
                xt_t[:, NLT // 2 :, :], xt8d.ap()[b, :, NLT // 2 :, :]
            )
            xb8_t = xb8_pool.tile([128, 2, 2, L], fp8, name="xb8_t", tag="xb8_t")
            nc.sync.dma_start(xb8_t[:], xb8d.ap()[b])
            xb16_t = xb16_pool.tile([128, NCC, L], bf16, name="xb16_t", tag="xb16_t")
            nc.sync.dma_start(xb16_t[:], xb16d.ap()[b])

            # --- mm1 (upper-triangle block-columns only; E is symmetric) ---
            psc_sb = []
            r_ts = []
            diag_ts = []
            eblk_sb = {}  # (dc, m) -> SBUF copy of E[dc][:, m-block]
            for m in range(NCC):
                e_t = e_psum.tile([128, C], f32, name="e_t", tag="e_t")
                mm0 = None
                for i in range(NDR):
                    mm = nc.tensor.matmul(
                        e_t[:, m * 128 :],
                        lhsT=xt_t[:, 2 * i : 2 * i + 2, m * 128 : (m + 1) * 128],
                        rhs=xt_t[:, 2 * i : 2 * i + 2, m * 128 :],
                        perf_mode=DR,
                        start=(i == 0),
                        stop=(i == NDR - 1),
                    )
                    if i == 0:
                        mm0 = mm
                # fill columns [0:m*128] by transposing earlier chunks' blocks.
                # start=False so the accumulation group's has_written clear is
                # not re-triggered; the explicit dep keeps each transpose after
                # that group's first matmul.
                for dc in range(m):
                    tr = nc.tensor.matmul(
                        e_t[:, dc * 128 : (dc + 1) * 128],
                        lhsT=eblk_sb.pop((dc, m))[:],
                        rhs=identity_f[:],
                        is_transpose=True,
                        start=False,
                        stop=True,
                        skip_group_check=True,
                    )
                    tile.add_dep_helper(
                        tr.ins, mm0.ins, reason="transpose after bank clear"
                    )
                # stage upper blocks needed by later chunks before e_t is freed
                for mc in range(m + 1, NCC):
                    blk = eblk_pool.tile([128, 128], f32, name="eblk", tag="eblk")
                    nc.vector.tensor_copy(blk[:], e_t[:, mc * 128 : (mc + 1) * 128])
                    eblk_sb[(m, mc)] = blk
                m_t = st_pool.tile([128, 1], f32)
                nc.vector.tensor_reduce(m_t[:], e_t[:], axis=AX.X, op=OP.min)
                p_t = prow_pool.tile([128, C], bf16)
                s_t = st_pool.tile([128, 1], f32)
                nc.scalar.activation(
                    p_t[:], e_t[:], ACT.Exp, bias=m_t[:], scale=-1.0, accum_out=s_t[:]
                )
                r_t = st_pool.tile([128, 1], f32, name="r_t", tag="r_t", bufs=8)
                nc.vector.reciprocal(r_t[:], s_t[:])
                d_t = st_pool.tile([128, 128], bf16, name="d_t", tag="d_t", bufs=8)
                nc.vector.tensor_scalar_mul(d_t[:], identity[:], s_t[:])
                r_ts.append(r_t)
                diag_ts.append(d_t)
                psc_sb.append(p_t)
                # interleave one mm2 m-chunk of the previous batch between
                # mm1 chunks so the PE fills epilogue-drain gaps
                if prev is not None:
                    emit_mm2_quarter(prev, m)

            # --- transpose P -> PT tiles [128 d, 2, C], scaled by gamma, fp8 ---
            pt_sb = [
                pt_pool.tile([128, 2, C], fp8, name="pt_sb", tag="pt_sb")
                for _ in range(NCC // 2)
            ]
            for m in range(NCC):
                for i in range(NCC):
                    tp = t_psum.tile([128, 128], bf16)
                    nc.tensor.transpose(
                        tp[:], psc_sb[m][:, i * 128 : (i + 1) * 128], identity[:]
                    )
                    dst = pt_sb[i // 2][:, i % 2, m * 128 : (m + 1) * 128]
                    nc.scalar.activation(dst, tp[:], ACT.Copy, scale=gamma_bc[:])

            o_sb = [
                out_pool.tile([128, L], bf16, name="o_t", tag="o_t")
                for _ in range(NCC)
            ]
            prev = (b, pt_sb, diag_ts, r_ts, xb8_t, xb16_t, o_sb)

        # pipeline tail: mm2 of the last batch
        for m in range(NCC):
            emit_mm2_quarter(prev, m, tail=True)

    nc.compile()
    return nc


def _get_nc():
    if "nc" not in _CACHE:
        _CACHE["nc"] = build_nc(BPC)
    return _CACHE["nc"]


def _prep_inputs(x: np.ndarray, gamma: np.ndarray):
    x = np.ascontiguousarray(np.asarray(x, dtype=np.float32))
    gamma = np.asarray(gamma, dtype=np.float32).reshape(1, 1)
    # partition-major swizzles so every device DMA load is contiguous:
    # xb16 [b, p, m, l] with c = m*128 + p
    xb16 = np.ascontiguousarray(
        x.reshape(B, 4, 128, L).transpose(0, 2, 1, 3)
    ).astype(ml_dtypes.bfloat16)
    # xb8 [b, p, i, j, l] with c = i*256 + j*128 + p  (DoubleRow k-pairing)
    xb8 = np.ascontiguousarray(
        x.reshape(B, 2, 2, 128, L).transpose(0, 3, 1, 2, 4)
    ).astype(FP8_NP)
    # xt8 [b, p, n, c] with l = n*128 + p
    xt8 = np.ascontiguousarray(
        x.transpose(0, 2, 1).reshape(B, L // 128, 128, C).transpose(0, 2, 1, 3)
    ).astype(FP8_NP)
    in_maps = []
    for c in range(N_CORES):
        sl = slice(c * BPC, (c + 1) * BPC)
        in_maps.append(
            {
                "xt8": np.ascontiguousarray(xt8[sl]),
                "xb8": np.ascontiguousarray(xb8[sl]),
                "xb16": np.ascontiguousarray(xb16[sl]),
                "gamma": gamma,
            }
        )
    return in_maps


def kernel(x: np.ndarray, gamma: np.ndarray) -> np.ndarray:
    from concourse.bass_utils import run_bass_kernel_spmd

    nc = _get_nc()
    in_maps = _prep_inputs(x, gamma)
    res = run_bass_kernel_spmd(nc, in_maps, core_ids=list(range(N_CORES)))
    y = np.concatenate([res.results[c]["y"] for c in range(N_CORES)], axis=0)
    return y.astype(np.float32)


def _make_exec_jit(nc, in_specs_names, out_shape, out_dtype=None):
    """One-bass_exec jit over 8 cores, mirroring run_bass_via_pjrt."""
    import jax
    from jax.sharding import Mesh, PartitionSpec
    from jax.experimental.shard_map import shard_map
    from concourse.bass2jax import (
        _bass_exec_p,
        install_neuronx_cc_hook,
        partition_id_tensor,
    )

    install_neuronx_cc_hook()
    out_aval = jax.core.ShapedArray(out_shape, out_dtype or np.float32)
    out_name = in_specs_names[-1]

    def body(*args):
        outs = _bass_exec_p.bind(
            *args,
            partition_id_tensor(),
            out_avals=(out_aval,),
            in_names=tuple(in_specs_names) + ("partition_id",),
            out_names=(out_name,),
            lowering_input_output_aliases=(),
            sim_require_finite=True,
            sim_require_nnan=True,
            nc=nc,
        )
        return outs[0]

    mesh = Mesh(np.asarray(jax.devices()[:N_CORES]), ("core",))
    spec = PartitionSpec("core")
    jitted = jax.jit(
        shard_map(
            body,
            mesh=mesh,
            in_specs=(spec,) * len(in_specs_names),
            out_specs=spec,
            check_rep=False,
        ),
        keep_unused=True,
    )
    sharding = jax.sharding.NamedSharding(mesh, spec)
    return jitted, sharding


def _build_tiny_nc():
    """Minimal kernel with the same call structure, for dispatch-floor calibration."""
    import concourse.tile as tile
    from concourse import bacc, mybir

    f32 = mybir.dt.float32
    nc = bacc.Bacc("TRN2", target_bir_lowering=False, debug=False, num_devices=N_CORES)
    ad = nc.dram_tensor("a", [128, 128], f32, kind="ExternalInput")
    bd = nc.dram_tensor("bout", [128, 128], f32, kind="ExternalOutput")
    with tile.TileContext(nc) as tc:
        with tc.tile_pool(name="p", bufs=1) as pool:
            t = pool.tile([128, 128], f32)
            nc.sync.dma_start(t[:], ad.ap())
            nc.sync.dma_start(bd.ap(), t[:])
    nc.compile()
    return nc


def measure_hw_time(x: np.ndarray, gamma: np.ndarray, calls: int = 30, reps: int = 5):
    """Estimate per-NEFF device time: loop a cached jit on device-resident
    inputs, subtract the dispatch floor measured with a near-empty kernel.

    Returns (exec_ns_estimate, per_call_big_ns, per_call_tiny_ns)."""
    import time

    import jax

    nc = _get_nc()
    in_maps = _prep_inputs(x, gamma)

    names = ["xt8", "xb8", "xb16", "gamma", "y"]
    jit_big, sh = _make_exec_jit(nc, names, (BPC, C, L), ml_dtypes.bfloat16)
    args = [
        np.concatenate([m[k] for m in in_maps], axis=0) for k in names[:-1]
    ] + [np.zeros((B, C, L), ml_dtypes.bfloat16)]
    big_args = [jax.device_put(a, sh) for a in args]

    tiny = _CACHE.get("tiny_nc")
    if tiny is None:
        tiny = _CACHE["tiny_nc"] = _build_tiny_nc()
    jit_tiny, sh2 = _make_exec_jit(tiny, ["a", "bout"], (128, 128))
    a_g = np.zeros((N_CORES * 128, 128), np.float32)
    tiny_args = [jax.device_put(a, sh2) for a in (a_g, np.zeros_like(a_g))]

    jax.block_until_ready(jit_big(*big_args))
    jax.block_until_ready(jit_tiny(*tiny_args))

    def per_call(f, args):
        best = np.inf
        for _ in range(reps):
            t0 = time.perf_counter()
            for _ in range(calls):
                out = f(*args)
            jax.block_until_ready(out)
            best = min(best, (time.perf_counter() - t0) / calls)
        return best * 1e9

    t_tiny = per_call(jit_tiny, tiny_args)
    t_big = per_call(jit_big, big_args)
    return t_big - t_tiny, t_big, t_tiny


if __name__ == "__main__":
    rng = np.random.default_rng(0)
    x = rng.standard_normal((B, C, L), dtype=np.float32)
    gamma = np.zeros((1,), np.float32)
    y = kernel(x, gamma)
    err = np.abs(y - x).max() / np.abs(x).max()
    print("gamma=0 rel err (bf16 roundtrip expected):", err)
    ns, t1, t0 = measure_hw_time(x, gamma)
    print(f"HW exec time: {ns:.0f} ns  (single-call wall {t1:.0f} ns)")
